# revision 10
# baseline (speedup 1.0000x reference)
"""Trainium2 Bass kernel for nn_MetricSelfAttention.

Math: the reference's softmax is dead code, so
    nudged = (p1 @ M @ p2^T) @ p1
reassociates to
    nudged = p1 @ (M @ (p2^T @ p1))        (per-head 64x64 Gram matrix G)
collapsing the O(W^2) attention matrices entirely.  The kernel is then
memory-bound: per core it reads x1[b], x2[b] (8 MiB) and writes a partial
mixer product (4 MiB).

Sharding: 8 cores = 2 batches x 4 head-pairs.  Core (b, hg) computes heads
{2hg, 2hg+1} of batch b and the partial output
    out_partial = nudged[:, 128hg:128hg+128] @ W_mixer[:, 128hg:128hg+128].T
The host sums the 4 partials per batch and adds b_mixer.

LayerNorm handling:
  - gamma is folded into the projection on the host; a nonzero beta enters as
    a rank-1 bias applied with K=1 matmuls (beta @ P'); when beta == 0 the
    bias matmuls are omitted entirely (fast path).
  - x1 is normalized on-chip (DVE tensor_scalar), then PE-transposed to give
    x1n^T for the c-contraction projections.
  - x2 is never normalized or transposed: with x2n = rstd2*x2 - mu2*rstd2
    rowwise,
      F := x2n^T @ p1  ==  x2^T @ (rstd2 * p1) - 1 (x) ((mu2*rstd2)^T @ p1)
    so F comes from raw x2 (natural layout, w-contraction, computed as F^T
    with a 512-wide moving operand) plus a rank-1 correction.

All large matmul groups (x1 transposes, p1^T projection, F^T, nudged^T,
mixer) run the PE in float32r mode (1 cycle/row at N>=256 vs 4 for fp32).
fp32r operands must be rounded by their producer, so the PSUM->SBUF copies
write fp32r tiles, and x2 is DMA-cast to fp32r on load (SWDGE).
"""

from contextlib import ExitStack

import numpy as np

import concourse.bacc as bacc
import concourse.bass as bass
import concourse.tile as tile
from concourse import mybir
from concourse.bass_utils import run_bass_kernel_spmd
from concourse.masks import make_identity

B, W, C, N, K = 2, 2048, 512, 8, 64
NCORES = 8
HPC = 2          # heads per core
K2 = HPC * K     # 128 channels per core
EPS = 1e-5
FP32 = mybir.dt.float32
FP32R = mybir.dt.float32r
AF = mybir.ActivationFunctionType
OP = mybir.AluOpType

NT = W // 128    # 16 w-tiles
NQ = W // 512    # 4 w-quads
NJ = C // 128    # 4 c-chunks


def _body(ctx: ExitStack, tc: tile.TileContext, x1d, x2d, projd, mmatd,
          wmixd, outd, pbiasd):
    nc = tc.nc
    with_pbias = pbiasd is not None

    persist = ctx.enter_context(tc.tile_pool(name="persist", bufs=1))
    xpool = ctx.enter_context(tc.tile_pool(name="xload", bufs=3))
    xnpool = ctx.enter_context(tc.tile_pool(name="xn", bufs=6))
    spool = ctx.enter_context(tc.tile_pool(name="stats", bufs=8))
    outpool = ctx.enter_context(tc.tile_pool(name="outstage", bufs=2))
    ps_tp = ctx.enter_context(tc.tile_pool(name="ps_tp", bufs=2, space="PSUM"))
    ps_mm = ctx.enter_context(tc.tile_pool(name="ps_mm", bufs=2, space="PSUM"))
    ps_mo = ctx.enter_context(tc.tile_pool(name="ps_mo", bufs=2, space="PSUM"))

    # ---- constants / params -------------------------------------------------
    proj_s = persist.tile([128, NJ, K2], FP32)
    nc.sync.dma_start(out=proj_s, in_=projd.rearrange("(j p) k -> p j k", p=128))
    proj_r = persist.tile([128, NJ, K2], FP32R)
    nc.vector.tensor_copy(out=proj_r, in_=proj_s)
    wmix_s = persist.tile([K2, C], FP32)
    nc.sync.dma_start(out=wmix_s, in_=wmixd)
    wmix_r = persist.tile([K2, C], FP32R)
    nc.vector.tensor_copy(out=wmix_r, in_=wmix_s)
    mmat_s = persist.tile([K, HPC, K], FP32)
    nc.sync.dma_start(out=mmat_s, in_=mmatd)
    pbias_s = None
    if with_pbias:
        pbias_s = persist.tile([1, K2], FP32)
        nc.sync.dma_start(out=pbias_s, in_=pbiasd)

    neg_ones512 = persist.tile([1, 512], FP32)
    nc.vector.memset(neg_ones512, -1.0)
    eps_s = persist.tile([128, 1], FP32)
    nc.vector.memset(eps_s, EPS)
    ident = persist.tile([128, 128], FP32)
    make_identity(nc, ident)
    ident_r = persist.tile([128, 128], FP32R)
    nc.vector.tensor_copy(out=ident_r, in_=ident)
    if with_pbias:
        ones512 = persist.tile([1, 512], FP32)
        nc.vector.memset(ones512, 1.0)
        ones_col = persist.tile([128, 1], FP32)
        nc.vector.memset(ones_col, 1.0)

    # ---- persistent activations --------------------------------------------
    x1nT = persist.tile([128, NJ, W], FP32R)    # [c%128, c//128, w], rounded
    x2r_s = persist.tile([128, NT, C], FP32R)   # raw x2 (rounded on DMA load)
    rstd2_s = persist.tile([128, NT], FP32)     # per-row 1/std of x2
    pmr2_s = persist.tile([128, NT], FP32)      # per-row +mu*rstd of x2
    p1n_s = persist.tile([128, NT, K2], FP32)   # p1 natural
    p1s_s = persist.tile([128, NT, K2], FP32R)  # rstd2 * p1 (rowwise)
    p1T_s = persist.tile([K2, W], FP32R)        # p1^T
    ft_s = persist.tile([K2, C], FP32R)         # F^T = p1s^T x2 - wrow (x) 1
    f_s = persist.tile([128, NJ, K2], FP32)     # F (c on partitions)
    wrow_s = persist.tile([1, K2], FP32)        # (mu2*rstd2)^T @ p1
    g_s = persist.tile([K, HPC, K], FP32)       # per-head Gram
    h_bd_s = persist.tile([K2, K2], FP32R)      # block-diag H = M @ G
    nudgT_s = persist.tile([K2, W], FP32R)      # nudged^T
    if with_pbias:
        s1_s = persist.tile([1, K2], FP32)      # column sums of p1

    def stats_tile(sub, rstd_out):
        """bn stats for one [128, 512] tile -> rstd written, mv returned."""
        stats = spool.tile([128, 6], FP32, tag="bst")
        nc.vector.bn_stats(stats, sub)
        mv = spool.tile([128, 2], FP32, tag="mv")
        nc.vector.bn_aggr(mv, stats)
        std = spool.tile([128, 1], FP32, tag="std")
        nc.scalar.activation(std, mv[:, 1:2], AF.Sqrt, bias=eps_s, scale=1.0)
        nc.vector.reciprocal(rstd_out, std)
        return mv

    # ---- x1: load, LN (DVE normalize -> fp32r), PE-transpose -> x1nT -------
    for q in range(NQ):
        xq = xpool.tile([128, 4, C], FP32, tag="x1q")
        nc.sync.dma_start(
            out=xq, in_=x1d[q * 512:(q + 1) * 512, :].rearrange(
                "(t p) c -> p t c", p=128))
        xns = []
        for t in range(4):
            sub = xq[:, t, :]
            rstd = spool.tile([128, 1], FP32, tag="rstd1")
            mv = stats_tile(sub, rstd)
            xn = xnpool.tile([128, C], FP32R, tag="x1n")
            nc.vector.tensor_scalar(
                out=xn, in0=sub, scalar1=mv[:, 0:1], scalar2=rstd,
                op0=OP.subtract, op1=OP.mult)
            xns.append(xn)
        for j in range(NJ):
            ps = ps_tp.tile([128, 512], FP32R, tag="tp")
            for t in range(4):
                nc.tensor.transpose(
                    ps[:, t * 128:(t + 1) * 128],
                    xns[t][:, j * 128:(j + 1) * 128], ident_r)
            if j % 2 == 0:
                nc.scalar.copy(out=x1nT[:, j, q * 512:(q + 1) * 512], in_=ps)
            else:
                nc.vector.tensor_copy(out=x1nT[:, j, q * 512:(q + 1) * 512], in_=ps)

    # ---- x2: load raw with fp32r DMA-cast (persist), stats only ------------
    for q in range(NQ):
        nc.gpsimd.dma_start(
            out=x2r_s[:, 4 * q:4 * (q + 1), :],
            in_=x2d[q * 512:(q + 1) * 512, :].rearrange("(t p) c -> p t c", p=128))
        for t in range(4):
            tt = 4 * q + t
            mv = stats_tile(x2r_s[:, tt, :], rstd2_s[:, tt:tt + 1])
            nc.vector.tensor_mul(pmr2_s[:, tt:tt + 1], mv[:, 0:1],
                                 rstd2_s[:, tt:tt + 1])

    # ---- p1^T = P'^T @ x1n^T (+ pbias (x) ones) ----------------------------
    for q in range(NQ):
        pt = ps_mm.tile([128, 512], FP32, tag="mm")
        for j in range(NJ):
            nc.tensor.matmul(pt, lhsT=proj_r[:, j, :],
                             rhs=x1nT[:, j, q * 512:(q + 1) * 512],
                             start=(j == 0), stop=(j == NJ - 1) and not with_pbias)
        if with_pbias:
            nc.tensor.matmul(pt, lhsT=pbias_s, rhs=ones512, start=False, stop=True)
        if q % 2 == 0:
            nc.vector.tensor_copy(out=p1T_s[:, q * 512:(q + 1) * 512], in_=pt)
        else:
            nc.scalar.copy(out=p1T_s[:, q * 512:(q + 1) * 512], in_=pt)

    # ---- p1 natural = PE-transpose of p1^T ---------------------------------
    for t in range(NT):
        ps = ps_tp.tile([128, 512], FP32R, tag="tp")
        nc.tensor.transpose(ps[:, :K2], p1T_s[:, t * 128:(t + 1) * 128], ident_r)
        nc.scalar.copy(out=p1n_s[:, t, :], in_=ps[:, :K2])

    # ---- p1s = rstd2 * p1 (rowwise, -> fp32r), for the F^T matmul ----------
    for t in range(NT):
        nc.vector.tensor_scalar_mul(p1s_s[:, t, :], p1n_s[:, t, :],
                                    rstd2_s[:, t:t + 1])

    # ---- wrow = (mu2*rstd2)^T @ p1 -----------------------------------------
    wp = ps_mm.tile([128, 512], FP32, tag="mm")
    wpv = wp[:1, :K2]
    for t in range(NT):
        nc.tensor.matmul(wpv, lhsT=pmr2_s[:, t:t + 1], rhs=p1n_s[:, t, :],
                         start=(t == 0), stop=(t == NT - 1))
    nc.vector.tensor_copy(out=wrow_s, in_=wpv)

    # ---- F^T = p1s^T @ x2 - wrow (x) 1  ------------------------------------
    ftp = ps_mm.tile([128, 512], FP32, tag="mm")
    for t in range(NT):
        nc.tensor.matmul(ftp, lhsT=p1s_s[:, t, :], rhs=x2r_s[:, t, :],
                         start=(t == 0), stop=False)
    nc.tensor.matmul(ftp, lhsT=wrow_s, rhs=neg_ones512, start=False, stop=True)
    nc.vector.tensor_copy(out=ft_s, in_=ftp)

    # ---- F = PE-transpose of F^T (c on partitions) -------------------------
    fjp = ps_tp.tile([128, 4, 128], FP32R, tag="tp")
    for j in range(NJ):
        nc.tensor.transpose(fjp[:, j, :], ft_s[:, j * 128:(j + 1) * 128], ident_r)
    nc.scalar.copy(out=f_s, in_=fjp)

    # ---- s1 = column sums of p1 (beta rank-1 term in G) --------------------
    if with_pbias:
        sp = ps_mm.tile([128, 512], FP32, tag="mm")
        spv = sp[:1, :K2]
        for t in range(NT):
            nc.tensor.matmul(spv, lhsT=ones_col, rhs=p1n_s[:, t, :],
                             start=(t == 0), stop=(t == NT - 1))
        nc.vector.tensor_copy(out=s1_s, in_=spv)

    # ---- G_h = P'_h^T @ F_h (+ pbias_h (x) s1_h) ---------------------------
    gp = ps_mm.tile([128, 512], FP32, tag="mm")
    gpv = gp[:K, :HPC * K].rearrange("p (h k) -> p h k", h=HPC)
    for h in range(HPC):
        for j in range(NJ):
            nc.tensor.matmul(gpv[:, h, :],
                             lhsT=proj_s[:, j, h * K:(h + 1) * K],
                             rhs=f_s[:, j, h * K:(h + 1) * K],
                             start=(j == 0),
                             stop=(j == NJ - 1) and not with_pbias)
        if with_pbias:
            nc.tensor.matmul(gpv[:, h, :], lhsT=pbias_s[:, h * K:(h + 1) * K],
                             rhs=s1_s[:, h * K:(h + 1) * K],
                             start=False, stop=True)
    nc.vector.tensor_copy(out=g_s, in_=gpv)

    # ---- H_h = M_h @ G_h  (M symmetric so lhsT = M_h) ----------------------
    hp = ps_mm.tile([128, 512], FP32, tag="mm")
    hpv = hp[:, :K]
    for h in range(HPC):
        nc.tensor.matmul(hpv[h * K:(h + 1) * K, :], lhsT=mmat_s[:, h, :],
                         rhs=g_s[:, h, :])
    # pack as block-diagonal [K2, K2] so nudged^T is one base-0 matmul
    # (memset can't write fp32r; zero via x*0 from an fp32r source instead)
    nc.vector.tensor_scalar_mul(h_bd_s, ident_r, 0.0)
    for h in range(HPC):
        nc.vector.tensor_copy(out=h_bd_s[h * K:(h + 1) * K, h * K:(h + 1) * K],
                              in_=hpv[h * K:(h + 1) * K, :])

    # ---- nudged^T = H_bd^T @ p1^T ------------------------------------------
    for q in range(NQ):
        ntp = ps_mm.tile([128, 512], FP32, tag="mm")
        nc.tensor.matmul(ntp, lhsT=h_bd_s,
                         rhs=p1T_s[:, q * 512:(q + 1) * 512])
        nc.scalar.copy(out=nudgT_s[:, q * 512:(q + 1) * 512], in_=ntp)

    # ---- mixer partial: out = nudged @ Wmix_slice --------------------------
    for q in range(NQ):
        stage = outpool.tile([128, 4, C], FP32, tag="ostage")
        for t in range(4):
            w_t = q * 4 + t
            mo = ps_mo.tile([128, C], FP32, tag="mo")
            nc.tensor.matmul(mo, lhsT=nudgT_s[:, w_t * 128:(w_t + 1) * 128],
                             rhs=wmix_r)
            if t % 2 == 0:
                nc.vector.tensor_copy(out=stage[:, t, :], in_=mo)
            else:
                nc.scalar.copy(out=stage[:, t, :], in_=mo)
        nc.sync.dma_start(
            out=outd[q * 512:(q + 1) * 512, :].rearrange("(t p) c -> p t c", p=128),
            in_=stage)


_PROGRAM_CACHE = {}


def _get_program(with_pbias: bool):
    key = ("nc", with_pbias)
    if key in _PROGRAM_CACHE:
        return _PROGRAM_CACHE[key]
    nc = bacc.Bacc("TRN2", debug=False, num_devices=NCORES)
    x1d = nc.dram_tensor("x1", [W, C], FP32, kind="ExternalInput").ap()
    x2d = nc.dram_tensor("x2", [W, C], FP32, kind="ExternalInput").ap()
    projd = nc.dram_tensor("proj", [C, K2], FP32, kind="ExternalInput").ap()
    mmatd = nc.dram_tensor("mmat", [K, HPC, K], FP32, kind="ExternalInput").ap()
    wmixd = nc.dram_tensor("wmix", [K2, C], FP32, kind="ExternalInput").ap()
    pbiasd = None
    if with_pbias:
        pbiasd = nc.dram_tensor("pbias", [1, K2], FP32, kind="ExternalInput").ap()
    outd = nc.dram_tensor("out", [W, C], FP32, kind="ExternalOutput").ap()
    with tile.TileContext(nc) as tc:
        with ExitStack() as ctx:
            _body(ctx, tc, x1d, x2d, projd, mmatd, wmixd, outd, pbiasd)
    nc.compile()
    _PROGRAM_CACHE[key] = nc
    return nc


def _host_prep(inputs):
    x1 = np.ascontiguousarray(np.asarray(inputs["x1"], np.float32))
    x2 = np.ascontiguousarray(np.asarray(inputs["x2"], np.float32))
    gamma = np.asarray(inputs["gamma"], np.float32)
    beta = np.asarray(inputs["beta"], np.float32)
    proj = np.asarray(inputs["proj_nck"], np.float32)
    halves = np.asarray(inputs["halves"], np.float32)
    diagonals = np.asarray(inputs["diagonals"], np.float32)
    wmix = np.asarray(inputs["W_mixer"], np.float32)

    iu0, iu1 = np.triu_indices(K, k=1)
    m = np.zeros((N, K, K), np.float32)
    m[:, iu0, iu1] = halves
    m = m + np.swapaxes(m, -1, -2)
    d = np.arange(K)
    m[:, d, d] = diagonals

    pgam = proj * gamma[None, :, None]          # gamma folded into projection
    with_pbias = bool(np.any(beta))
    pbias = np.einsum("c,nck->nk", beta, pgam) if with_pbias else None

    in_maps = []
    for core in range(NCORES):
        b, hg = divmod(core, NCORES // B)
        h0 = HPC * hg
        im = {
            "x1": x1[b],
            "x2": x2[b],
            "proj": np.ascontiguousarray(
                np.concatenate([pgam[h0 + i] for i in range(HPC)], axis=1)),
            "mmat": np.ascontiguousarray(
                np.stack([m[h0 + i] for i in range(HPC)], axis=1)),
            "wmix": np.ascontiguousarray(
                wmix[:, K2 * hg:K2 * (hg + 1)].T),
        }
        if with_pbias:
            im["pbias"] = np.ascontiguousarray(
                np.concatenate([pbias[h0 + i] for i in range(HPC)])[None, :])
        in_maps.append(im)
    return in_maps, with_pbias


def kernel(**inputs) -> np.ndarray:
    in_maps, with_pbias = _host_prep(inputs)
    nc = _get_program(with_pbias)
    res = run_bass_kernel_spmd(nc, in_maps, core_ids=list(range(NCORES)))
    out = np.zeros((B, W, C), np.float32)
    for core in range(NCORES):
        b = core // (NCORES // B)
        out[b] += res.results[core]["out"]
    out += np.asarray(inputs["b_mixer"], np.float32)[None, None, :]
    return out
